# revision 20
# baseline (speedup 1.0000x reference)
"""MemN2N (3-hop memory network) forward pass on 8 Trainium2 NeuronCores.

Strategy (per spec sharding hint):
 - Data-parallel over batch (4 batches/core) for the embedding gathers and
   hop math. Embedding tables are replicated in each core's DRAM as bf16.
 - Position encoding pe[l,e] = 1 + a_l*b_e is rank-2, so each embedding
   reduction over words L needs only two weighted sums (plain and a-weighted),
   computed on the TensorEngine with a block-diagonal weight matrix
   (K = 4 batches x 32 word-slots).
 - After the hops, u3 (4,256) is AllGathered; each core then computes the
   final linear for its 4000-vocab shard (W^T pre-sliced/transposed on host),
   BatchNorm over the full batch (local per vocab column), and a
   log-softmax whose log-sum-exp is combined across cores via a second tiny
   AllGather of per-core per-batch lse values.
 - Head layout is [vocab-chunk partitions (125) x 32 chunks, batch(32) free]
   so BN stats are free-dim reductions and the affine is per-partition.

kernel(**inputs) takes the FULL unsharded inputs and returns the FULL
(32, 32000) float32 output.
"""
import sys
sys.path.insert(0, '/opt/trn_rl_repo')
import numpy as np
import ml_dtypes
from contextlib import ExitStack

import concourse.bass as bass
import concourse.bacc as bacc
import concourse.tile as tile
from concourse import mybir
from concourse.alu_op_type import AluOpType
from concourse.bass_utils import run_bass_kernel_spmd

F32 = mybir.dt.float32
BF16 = mybir.dt.bfloat16
I16 = mybir.dt.int16
AF = mybir.ActivationFunctionType
AX = mybir.AxisListType

# problem constants
B, M, L, LQ, E, V = 32, 100, 30, 30, 256, 32000
NC_ = 8
BL = B // NC_          # 4 local batches
VL = V // NC_          # 4000 local vocab
CH = 125               # head vocab-chunk size (32 chunks of 125 = 4000)
NCH = VL // CH         # 32
NQ = 8                 # gather chunk: 8 m-columns -> 1024 indices
EH = E // 2            # 128, e-half

_cache = {}


def _consts():
    a = (np.arange(1, L + 1, dtype=np.float64) - (L + 1) / 2.0)        # (30,)
    b = 4.0 * (np.arange(1, E + 1, dtype=np.float64) - (E + 1) / 2.0) / (E * L)
    wmat = np.zeros((128, 8), np.float32)
    for r in range(4):
        for l in range(L):
            wmat[32 * r + l, 2 * r + 0] = 1.0
            wmat[32 * r + l, 2 * r + 1] = a[l]
    bvec = b.astype(np.float32).reshape(2, EH).T.copy()               # [128,2]
    ones1 = np.ones((1, 128), np.float32)
    ident = np.eye(128, dtype=np.float32)
    return wmat, bvec, ones1, ident


def _wrap_idx(flat):
    """int16 flat index array (len % 16 == 0) -> [128, len//16] wrapped+tiled."""
    n = flat.shape[0]
    wr = flat.reshape(n // 16, 16).T.astype(np.int16)   # [16, n//16]
    return np.tile(wr, (8, 1)).copy()                   # [128, n//16]


def build_nc(repeat=1, debug=False, stage=5, hop_sub=9, nhop=3):
    nc = bacc.Bacc("TRN2", target_bir_lowering=False, debug=False,
                   num_devices=NC_)

    # ---- DRAM I/O ----
    tabs = [nc.dram_tensor(f"tab{t}", [V, E], BF16, kind="ExternalInput").ap()
            for t in range(4)]
    sidx = nc.dram_tensor("sidx", [128, M * 128 // 16], I16, kind="ExternalInput").ap()
    qidx = nc.dram_tensor("qidx", [128, 8], I16, kind="ExternalInput").ap()
    qmc = nc.dram_tensor("qmc", [128, 1], F32, kind="ExternalInput").ap()
    pmi = nc.dram_tensor("pmi", [1, BL * M], F32, kind="ExternalInput").ap()
    wti = nc.dram_tensor("wti", [2, 128, VL], BF16, kind="ExternalInput").ap()
    gbi = nc.dram_tensor("gbi", [128, 2, NCH], F32, kind="ExternalInput").ap()
    vmi = nc.dram_tensor("vmi", [128, NCH, B], F32, kind="ExternalInput").ap()
    wmi = nc.dram_tensor("wmi", [128, 8], BF16, kind="ExternalInput").ap()
    bvi = nc.dram_tensor("bvi", [128, 2], F32, kind="ExternalInput").ap()
    on1 = nc.dram_tensor("on1", [1, 128], F32, kind="ExternalInput").ap()
    idi = nc.dram_tensor("idi", [128, 128], F32, kind="ExternalInput").ap()
    out = nc.dram_tensor("out", [128, NCH, B], F32, kind="ExternalOutput").ap()
    dbg = None
    if debug:
        dbg = nc.dram_tensor("dbg", [128, 64], F32, kind="ExternalOutput").ap()

    u3_loc = nc.dram_tensor("u3_loc", [2, 128, BL], F32).ap()
    u3_gth = nc.dram_tensor("u3_gth", [NC_, 2, 128, BL], F32,
                            addr_space="Shared").ap()
    lse_loc = nc.dram_tensor("lse_loc", [B], F32).ap()
    lse_gth = nc.dram_tensor("lse_gth", [NC_, B], F32,
                             addr_space="Shared").ap()

    with tile.TileContext(nc) as tc, ExitStack() as ctx:
        cons = ctx.enter_context(tc.tile_pool(name="cons", bufs=1))
        embp = ctx.enter_context(tc.tile_pool(name="embp", bufs=1))
        rt_p = ctx.enter_context(tc.tile_pool(name="rt", bufs=3))
        tmp = ctx.enter_context(tc.tile_pool(name="tmp", bufs=4))
        up = ctx.enter_context(tc.tile_pool(name="up", bufs=2))
        pp_e = ctx.enter_context(tc.tile_pool(name="pp_e", bufs=2, space="PSUM"))
        pp_s = ctx.enter_context(tc.tile_pool(name="pp_s", bufs=1, space="PSUM"))
        pp_w = ctx.enter_context(tc.tile_pool(name="pp_w", bufs=1, space="PSUM"))
        pp_t = ctx.enter_context(tc.tile_pool(name="pp_t", bufs=2, space="PSUM"))

        # ---- constants / small inputs (loaded once) ----
        wmat = cons.tile([128, 8], BF16)
        nc.sync.dma_start(wmat[:], wmi)
        bvec = cons.tile([128, 2], F32)
        nc.sync.dma_start(bvec[:], bvi)
        ones1 = cons.tile([1, 128], F32)
        nc.sync.dma_start(ones1[:], on1)
        ident = cons.tile([128, 128], F32)
        nc.sync.dma_start(ident[:], idi)
        qm_sb = cons.tile([128, 1], F32)
        nc.sync.dma_start(qm_sb[:], qmc)
        pm_sb = cons.tile([1, BL * M], F32)
        nc.sync.dma_start(pm_sb[:], pmi)
        sidx_sb = cons.tile([128, M * 8], I16)
        nc.sync.dma_start(sidx_sb[:], sidx)
        qidx_sb = cons.tile([128, 8], I16)
        nc.sync.dma_start(qidx_sb[:], qidx)
        gb_sb = cons.tile([128, 2, NCH], F32)
        nc.sync.dma_start(gb_sb[:], gbi)
        wt_sb = cons.tile([128, 2, VL], BF16)
        nc.sync.dma_start(wt_sb[:, 0, :], wti[0])
        nc.sync.dma_start(wt_sb[:, 1, :], wti[1])
        logvm = cons.tile([128, NCH, B], F32)
        nc.sync.dma_start(logvm[:], vmi)
        # emb storage: 4 tables x [128, 2, M, 4, 2] f32
        emb = [embp.tile([128, 2, M, 4, 2], F32, tag=f"emb{t}", name=f"emb{t}")
               for t in range(4)]

        for rep in range(repeat):
            # pm-derived: pm1e30 = (pm-1)*1e30 ; computed per repeat (cheap)
            pm_m1 = tmp.tile([1, BL * M], F32, tag="pm_m1")
            nc.vector.tensor_scalar(pm_m1[:], pm_sb[:], -1.0, 1e30,
                                    AluOpType.add, AluOpType.mult)
            # logvm init (in-place would break repeat; compute into y later)
            # wmatq = wmat * qm  (bf16)
            wmatq = tmp.tile([128, 8], BF16, tag="wmatq")
            nc.vector.tensor_scalar_mul(wmatq[:], wmat[:], qm_sb[:, 0:1])

            # ---- query encode ----
            rq = rt_p.tile([128, 1, E], BF16, tag="rq")
            nc.gpsimd.dma_gather(rq[:], tabs[0], qidx_sb[:, :],
                                 num_idxs=128, num_idxs_reg=128, elem_size=E)
            ps_q = pp_s.tile([128, 16], F32, tag="scr")
            for h in range(2):
                nc.tensor.matmul(ps_q[:, h * 8:(h + 1) * 8],
                                 rq[:, 0, h * EH:(h + 1) * EH],
                                 wmatq[:], start=True, stop=True)
            q_sb = tmp.tile([128, 16], F32, tag="q_sb")
            nc.scalar.copy(q_sb[:], ps_q[:])
            u_cur = up.tile([128, 2, BL], F32, tag="u")
            for h in range(2):
                # u = plain + bvec * a_part
                psq_odd = bass.AP(q_sb.tensor, q_sb[:].offset + h * 8 + 1,
                                  [q_sb[:].ap[0], [2, BL]])
                psq_evn = bass.AP(q_sb.tensor, q_sb[:].offset + h * 8 + 0,
                                  [q_sb[:].ap[0], [2, BL]])
                nc.vector.scalar_tensor_tensor(
                    u_cur[:, h, :], psq_odd, bvec[:, h:h + 1], psq_evn,
                    AluOpType.mult, AluOpType.add)

            if stage >= 2:
              # ---- story gathers + PE reductions ----
              nchunks = (M + NQ - 1) // NQ
              for t in range(4):
                  for j in range(nchunks):
                    q0 = j * NQ
                    nq = min(NQ, M - q0)
                    rt = rt_p.tile([128, NQ, E], BF16, tag="rt")
                    nc.gpsimd.dma_gather(
                        rt[:, :nq, :], tabs[t],
                        sidx_sb[:, q0 * 8:(q0 + nq) * 8],
                        num_idxs=nq * 128, num_idxs_reg=nq * 128, elem_size=E)
                    ps_e = pp_e.tile([128, NQ * 16], F32, tag="pse")
                    for q in range(nq):
                        for h in range(2):
                            nc.tensor.matmul(
                                ps_e[:, q * 16 + h * 8: q * 16 + h * 8 + 8],
                                rt[:, q, h * EH:(h + 1) * EH],
                                wmat[:], start=True, stop=True)
                    # flush psum -> emb[t][:, h, q0:q0+nq, :, :]
                    for h in range(2):
                        src = bass.AP(ps_e.tensor, ps_e[:].offset + h * 8,
                                      [ps_e[:].ap[0], [16, nq], [1, 8]])
                        dst = bass.AP(emb[t].tensor,
                                      emb[t][:].offset + h * 800 + q0 * 8,
                                      [emb[t][:].ap[0], [8, nq], [1, 8]])
                        nc.scalar.copy(dst, src)

            # ---- hops ----
            for hop in range(nhop if stage >= 3 else 0):
                embA, embC = emb[hop], emb[hop + 1]
                # U2[p,h,r,s]: s=0 -> u, s=1 -> u*bvec
                u2 = tmp.tile([128, 2, BL, 2], F32, tag="u2")
                nc.vector.tensor_copy(
                    out=bass.AP(u2.tensor, u2[:].offset,
                                [u2[:].ap[0], [8, 2], [2, BL]]),
                    in_=u_cur[:])
                for h in range(2):
                    nc.vector.tensor_scalar_mul(
                        bass.AP(u2.tensor, u2[:].offset + h * 8 + 1,
                                [u2[:].ap[0], [2, BL]]),
                        u_cur[:, h, :], bvec[:, h:h + 1])
                # scores
                ps_sc = pp_s.tile([1, BL * M], F32, tag="scr")
                for r in range(BL):
                    k = 0
                    for h in range(2):
                        for s in range(2):
                            nc.tensor.matmul(
                                ps_sc[0:1, r * M:(r + 1) * M],
                                u2[:, h, r, s:s + 1],
                                bass.AP(embA.tensor,
                                        embA[:].offset + h * 800 + r * 2 + s,
                                        [embA[:].ap[0], [8, M]]),
                                start=(k == 0), stop=(k == 3))
                            k += 1
                if hop_sub < 2:
                    continue
                # masked softmax on [1, 400] (partition 0)
                sm = tmp.tile([1, BL * M], F32, tag="sm")
                nc.vector.tensor_mul(sm[:], ps_sc[:], pm_sb[:])
                nc.vector.tensor_add(sm[:], sm[:], pm_m1[:])
                mx = tmp.tile([1, BL], F32, tag="mx")
                nc.vector.tensor_reduce(
                    mx[:], bass.AP(sm.tensor, sm[:].offset,
                                   [sm[:].ap[0], [M, BL], [1, M]]),
                    AX.X, AluOpType.max, negate=True)
                ex = tmp.tile([1, BL * M], F32, tag="ex")
                for r in range(BL):
                    nc.scalar.activation(ex[0:1, r * M:(r + 1) * M],
                                         sm[0:1, r * M:(r + 1) * M],
                                         AF.Exp, bias=mx[0:1, r:r + 1], scale=1.0)
                nc.vector.tensor_mul(ex[:], ex[:], pm_sb[:])
                ssum = tmp.tile([1, BL], F32, tag="ssum")
                nc.vector.tensor_reduce(
                    ssum[:], bass.AP(ex.tensor, ex[:].offset,
                                     [ex[:].ap[0], [M, BL], [1, M]]),
                    AX.X, AluOpType.add)
                nc.vector.tensor_scalar_add(ssum[:], ssum[:], 1e-13)
                nc.vector.reciprocal(ssum[:], ssum[:])
                pw = tmp.tile([1, BL * M], F32, tag="pw")
                for r in range(BL):
                    nc.vector.tensor_scalar_mul(pw[0:1, r * M:(r + 1) * M],
                                                ex[0:1, r * M:(r + 1) * M],
                                                ssum[0:1, r:r + 1])
                if hop_sub < 3:
                    continue
                # broadcast p across partitions via K=1 matmul
                ps_p = pp_t.tile([128, BL * M], F32, tag="aux")
                nc.tensor.matmul(ps_p[:], ones1[:], pw[:], start=True, stop=True)
                if hop_sub < 4:
                    continue
                pb_sb = tmp.tile([128, BL * M], F32, tag="pb_sb")
                nc.scalar.copy(pb_sb[:], ps_p[:])
                if hop_sub < 5:
                    continue
                # o[p,h,r,s] = sum_m embC[p,h,m,r,s] * p[r,m]
                o_t = tmp.tile([128, 2, BL, 2], F32, tag="ot")
                scr = tmp.tile([128, 2, 2, M], F32, tag="scr")
                for h in range(2):
                    for r in range(BL):
                        for s in range(2):
                            nc.vector.tensor_mul(
                                scr[:, h, s, :],
                                bass.AP(embC.tensor,
                                        embC[:].offset + h * 800 + r * 2 + s,
                                        [embC[:].ap[0], [8, M]]),
                                pb_sb[:, r * M:(r + 1) * M])
                            nc.vector.tensor_reduce(
                                o_t[:, h, r, s:s + 1], scr[:, h, s, :],
                                AX.X, AluOpType.add)
                if hop_sub < 6:
                    continue
                # u' = u + o_plain + bvec*o_a
                u_nxt = up.tile([128, 2, BL], F32, tag="u")
                nc.vector.tensor_add(
                    u_nxt[:], u_cur[:],
                    bass.AP(o_t.tensor, o_t[:].offset,
                            [o_t[:].ap[0], [8, 2], [2, BL]]))
                for h in range(2):
                    nc.vector.scalar_tensor_tensor(
                        u_nxt[:, h, :],
                        bass.AP(o_t.tensor, o_t[:].offset + h * 8 + 1,
                                [o_t[:].ap[0], [2, BL]]),
                        bvec[:, h:h + 1],
                        u_nxt[:, h, :],
                        AluOpType.mult, AluOpType.add)
                u_cur = u_nxt

            # ---- AllGather u3 ----
            if stage < 4:
                out_sb0 = tmp.tile([128, NCH, B], F32, tag="outsb")
                nc.vector.memset(out_sb0[:], 0.0)
                nc.vector.tensor_copy(
                    out_sb0[:, 0, 0:8],
                    bass.AP(u_cur.tensor, u_cur[:].offset,
                            [u_cur[:].ap[0], [1, 8]]))
                if stage >= 2:
                    for t in range(4):
                        nc.vector.tensor_copy(
                            out_sb0[:, 1 + t, 0:32],
                            bass.AP(emb[t].tensor, emb[t][:].offset,
                                    [emb[t][:].ap[0], [800, 2], [1, 16]]))
                nc.sync.dma_start(out, out_sb0[:])
                continue
            for h in range(2):
                nc.sync.dma_start(u3_loc[h], u_cur[:, h, :])
            nc.gpsimd.collective_compute(
                "AllGather", AluOpType.bypass,
                replica_groups=[list(range(NC_))],
                ins=[u3_loc], outs=[u3_gth])
            u3g = tmp.tile([128, 2, B], F32, tag="u3g")
            # DRAM (c,h,p,r) -> SBUF [p, h, (c,r)]
            for h in range(2):
                src = bass.AP(u3_gth.tensor, h * 128 * BL,
                              [[BL, 128], [2 * 128 * BL, NC_], [1, BL]])
                nc.sync.dma_start(u3g[:, h, :], src)
            u3b = tmp.tile([128, 2, B], BF16, tag="u3b")
            nc.vector.tensor_copy(u3b[:], u3g[:])
            if stage < 5:
                out_sb0 = tmp.tile([128, NCH, B], F32, tag="outsb")
                nc.vector.memset(out_sb0[:], 0.0)
                nc.vector.tensor_copy(out_sb0[:, 0, 0:16],
                                      bass.AP(u3g.tensor, u3g[:].offset,
                                              [u3g[:].ap[0], [1, 16]]))
                nc.sync.dma_start(out, out_sb0[:])
                continue

            # ---- final matmul: wx[v,b] ----
            ps_wx = pp_w.tile([128, NCH, B], F32, tag="pswx")
            nc.vector.memset(ps_wx[:, :, :], 0.0)
            for ch in range(NCH):
                for h in range(2):
                    nc.tensor.matmul(
                        ps_wx[:CH, ch, :],
                        wt_sb[:, h, ch * CH:(ch + 1) * CH],
                        u3b[:, h, :],
                        start=(h == 0), stop=(h == 1))

            # ---- BatchNorm over batch (free dim) ----
            mean = tmp.tile([128, NCH], F32, tag="mean")
            nc.vector.tensor_reduce(mean[:], ps_wx[:], AX.X, AluOpType.add)
            nc.vector.tensor_scalar_mul(mean[:], mean[:], 1.0 / B)
            sq = tmp.tile([128, NCH, B], F32, tag="sq")
            nc.scalar.activation(sq[:], ps_wx[:], AF.Square)
            var = tmp.tile([128, NCH], F32, tag="var")
            nc.vector.tensor_reduce(var[:], sq[:], AX.X, AluOpType.add)
            nc.vector.tensor_scalar_mul(var[:], var[:], 1.0 / B)
            msq = tmp.tile([128, NCH], F32, tag="msq")
            nc.vector.tensor_mul(msq[:], mean[:], mean[:])
            nc.vector.tensor_sub(var[:], var[:], msq[:])
            rstd = tmp.tile([128, NCH], F32, tag="rstd")
            eps_t = tmp.tile([128, 1], F32, tag="eps")
            nc.vector.memset(eps_t[:], 1e-5)
            nc.scalar.activation(rstd[:], var[:], AF.Sqrt, bias=eps_t[:], scale=1.0)
            nc.vector.reciprocal(rstd[:], rstd[:])
            av = tmp.tile([128, NCH], F32, tag="av")
            nc.vector.tensor_mul(av[:], gb_sb[:, 0, :], rstd[:])
            bv = tmp.tile([128, NCH], F32, tag="bv")
            nc.vector.tensor_mul(bv[:], av[:], mean[:])
            nc.vector.tensor_sub(bv[:], gb_sb[:, 1, :], bv[:])
            # y = av*wx + bv + logvm
            y_all = tmp.tile([128, NCH, B], F32, tag="yall")
            for ch in range(NCH):
                nc.scalar.activation(y_all[:, ch, :], ps_wx[:, ch, :], AF.Identity,
                                     bias=bv[:, ch:ch + 1], scale=av[:, ch:ch + 1])
            nc.vector.tensor_add(y_all[:], y_all[:], logvm[:])

            # ---- local sum(exp(y)) over vocab shard ----
            es = tmp.tile([128, NCH, B], F32, tag="es")
            nc.scalar.activation(es[:], y_all[:], AF.Exp)
            xs = tmp.tile([128, B], F32, tag="xs")
            nc.vector.tensor_reduce(
                xs[:], bass.AP(es.tensor, es[:].offset,
                               [es[:].ap[0], [1, B], [B, NCH]]),
                AX.X, AluOpType.add)
            ps_tr = pp_t.tile([B, 128], F32, tag="aux")
            nc.tensor.transpose(ps_tr[:], xs[:], ident[:])
            s_loc = tmp.tile([B, 1], F32, tag="sloc")
            nc.vector.tensor_reduce(s_loc[:], ps_tr[:, :CH], AX.X, AluOpType.add)
            lse_l = tmp.tile([B, 1], F32, tag="lsel")
            nc.scalar.activation(lse_l[:], s_loc[:], AF.Ln)
            nc.sync.dma_start(lse_loc, lse_l[:, 0])
            nc.gpsimd.collective_compute(
                "AllGather", AluOpType.bypass,
                replica_groups=[list(range(NC_))],
                ins=[lse_loc], outs=[lse_gth])
            lse8 = tmp.tile([B, NC_], F32, tag="lse8")
            nc.sync.dma_start(lse8[:], bass.AP(lse_gth.tensor, 0,
                                               [[1, B], [B, NC_]]))
            mx8 = tmp.tile([B, 1], F32, tag="mx8")
            nc.vector.tensor_reduce(mx8[:], lse8[:], AX.X, AluOpType.max)
            z8 = tmp.tile([B, NC_], F32, tag="z8")
            nc.vector.tensor_scalar_sub(z8[:], lse8[:], mx8[:, 0:1])
            nc.scalar.activation(z8[:], z8[:], AF.Exp)
            s8 = tmp.tile([B, 1], F32, tag="s8")
            nc.vector.tensor_reduce(s8[:], z8[:], AX.X, AluOpType.add)
            nc.scalar.activation(s8[:], s8[:], AF.Ln)
            glse = tmp.tile([B, 1], F32, tag="glse")
            nc.vector.tensor_add(glse[:], mx8[:], s8[:])
            # broadcast glse over partitions: [B,1] -> [1,B] -> K=1 matmul
            ps_g1 = pp_t.tile([1, B], F32, tag="aux")
            nc.tensor.transpose(ps_g1[:], glse[:], ident[:B, :B])
            g_row = tmp.tile([1, B], F32, tag="grow")
            nc.scalar.copy(g_row[:], ps_g1[:])
            ps_gb = pp_t.tile([128, B], F32, tag="aux")
            nc.tensor.matmul(ps_gb[:], ones1[:], g_row[:], start=True, stop=True)
            out_sb = tmp.tile([128, NCH, B], F32, tag="outsb")
            for ch in range(NCH):
                nc.vector.tensor_sub(out_sb[:, ch, :], y_all[:, ch, :], ps_gb[:])
            nc.sync.dma_start(out, out_sb[:])

            if debug and rep == 0:
                dbg_sb = tmp.tile([128, 64], F32, tag="dbgsb")
                # [0:8] = u3 (h,r); [8:24] = emb1[p, h=0, q=0, r, s] etc.
                nc.vector.tensor_copy(dbg_sb[:, 0:8],
                                      bass.AP(u_cur.tensor, u_cur[:].offset,
                                              [u_cur[:].ap[0], [1, 8]]))
                nc.vector.tensor_copy(dbg_sb[:, 8:24],
                                      bass.AP(emb[0].tensor, emb[0][:].offset,
                                              [emb[0][:].ap[0], [800, 2], [1, 8]]))
                nc.vector.tensor_copy(dbg_sb[:, 24:40],
                                      bass.AP(emb[1].tensor, emb[1][:].offset,
                                              [emb[1][:].ap[0], [800, 2], [1, 8]]))
                nc.vector.tensor_copy(dbg_sb[:, 48:64],
                                      bass.AP(ps_wx.tensor, ps_wx[:].offset,
                                              [ps_wx[:].ap[0], [1, 16]]))
                nc.sync.dma_start(dbg, dbg_sb[:])

    nc.compile()
    return nc


def marshal(inputs):
    """FULL inputs -> per-core in_maps."""
    wmat, bvec, ones1, ident = _consts()
    trainS = np.asarray(inputs['trainS'])
    trainQ = np.asarray(inputs['trainQ'])
    trainVM = np.asarray(inputs['trainVM'], dtype=np.float32)
    trainPM = np.asarray(inputs['trainPM'], dtype=np.float32)
    trainQM = np.asarray(inputs['trainQM'], dtype=np.float32)
    tabs = [np.asarray(inputs[k], dtype=np.float32).astype(ml_dtypes.bfloat16)
            for k in ('A1', 'A2', 'A3', 'A4')]
    W = np.asarray(inputs['W'], dtype=np.float32)
    gamma = np.asarray(inputs['gamma'], dtype=np.float32)
    beta = np.asarray(inputs['beta'], dtype=np.float32)

    in_maps = []
    for c in range(NC_):
        rb = slice(BL * c, BL * (c + 1))
        vs = VL * c
        # story indices
        arr = np.zeros((M, BL, 32), np.int16)
        arr[:, :, :L] = trainS[rb].transpose(1, 0, 2)
        sidx = _wrap_idx(arr.reshape(-1))
        # query indices
        qa = np.zeros((BL, 32), np.int16)
        qa[:, :LQ] = trainQ[rb, 0, :]
        qidx = _wrap_idx(qa.reshape(-1))
        # query mask column
        qmc = np.zeros((128, 1), np.float32)
        for r in range(BL):
            qmc[32 * r:32 * r + LQ, 0] = trainQM[BL * c + r]
        # W^T slice: wt[h, p, v] = W[vs+v, 128h+p]
        wt = W[vs:vs + VL].T.reshape(2, 128, VL).astype(ml_dtypes.bfloat16)
        # gamma/beta: gb[p, 0, ch] = gamma[vs + ch*125 + p] (p < 125)
        gb = np.zeros((128, 2, NCH), np.float32)
        gslice = gamma[vs:vs + VL].reshape(NCH, CH).T    # [125, 32]
        bslice = beta[vs:vs + VL].reshape(NCH, CH).T
        gb[:CH, 0, :] = gslice
        gb[:CH, 1, :] = bslice
        # VM^T -> log(VM + 1e-13): computed on host? cheap mask transform;
        # do log on host to save a device pass (mask values, not data).
        vmt = np.zeros((128, NCH, B), np.float32)
        vmt[:CH] = np.log(trainVM[:, vs:vs + VL].astype(np.float64)
                          + 1e-13).astype(np.float32).T.reshape(NCH, CH, B).transpose(1, 0, 2)
        in_maps.append({
            'tab0': tabs[0], 'tab1': tabs[1], 'tab2': tabs[2], 'tab3': tabs[3],
            'sidx': sidx, 'qidx': qidx, 'qmc': qmc,
            'pmi': trainPM[rb].reshape(1, BL * M).copy(),
            'wti': wt, 'gbi': gb, 'vmi': vmt,
            'wmi': wmat.astype(ml_dtypes.bfloat16), 'bvi': bvec,
            'on1': ones1, 'idi': ident,
        })
    return in_maps


def unmarshal(results):
    outf = np.zeros((B, V), np.float32)
    for c in range(NC_):
        o = np.asarray(results[c]['out']).reshape(128, NCH, B)[:CH]
        outf[:, VL * c:VL * (c + 1)] = o.transpose(2, 1, 0).reshape(B, VL)
    return outf


def kernel(**inputs):
    if 'nc' not in _cache:
        _cache['nc'] = build_nc()
    nc = _cache['nc']
    in_maps = marshal(inputs)
    res = run_bass_kernel_spmd(nc, in_maps, list(range(NC_)))
    return unmarshal(res.results)
